# revision 15
# baseline (speedup 1.0000x reference)
"""Trainium2 Bass kernel for nn_Loss_343597383760.

Loss:
    scores = predicted_values[rel_idx, e1_idx, e2_idx]        # [N] gather
    sig    = sigmoid(scores)
    total  = sum(lab*sig + (1-lab)*(1-sig)) = neg + sum(w*sig),  w = 2*lab-1
    loss   = -total / ((1+neg)*N)

Sharding (expert-style, per relation): core c owns relations {2c, 2c+1} of
predicted_values ([2,4096,4096] f32 = 128 MiB per core). Host buckets the
262144 triplets by owning core and converts each to a flat element index into
the local shard.

Layout: indices are packed by label sign into a [128, 264] int32 plane:
positive-label triplets fill partitions [0,64), negative fill [64,128); pad
slots hold index TOTAL, which points at an appended 0.0 element of the pv
shard so a pad contributes sigmoid(0)=0.5 exactly.

Device (raw bass, no TileContext — the tile framework's entry barriers and
exit semaphore-clear storm cost ~5us on a ~10us kernel):
    HWDGE load of the idx plane (sync engine)
    one SWDGE indirect-DMA gather of all 33792 elements (desc-gen is ~1.1us
    fixed per instruction with near-zero marginal per-descriptor cost)
    one ACT sigmoid with row-sum accumulation -> out [128, 1]
    HWDGE store of out
A dummy warm-up activation at the head of the Scalar stream forces the
auto-inserted ACT_TABLE_LOAD to run during the idx-load latency instead of
after the gather. Host combines: sum w*sig(s) = sum_{p<64} out[p]
- sum_{p>=64} out[p] - 0.5*pad_pos + 0.5*pad_neg.
"""

import numpy as np

import concourse.bass as bass
import concourse.bacc as bacc
from concourse import mybir
from concourse.bass_utils import run_bass_kernel_spmd

R, E, N = 16, 4096, 262144
NCORES = 8
RPC = R // NCORES            # relations per core
TOTAL = RPC * E * E          # elements in one core's shard
P = 128                      # SBUF partitions
COLS = 264                   # capacity per core = 128*264 = 33792
CAP = P * COLS
HALF = P // 2                # partitions per sign region
RCAP = HALF * COLS           # per-sign region capacity (16896)

# Set by test harness to capture a neuron-profile trace.
TRACE = False
LAST_RESULTS = None

_NC = None


def _build_nc():
    f32 = mybir.dt.float32
    i32 = mybir.dt.int32
    nc = bacc.Bacc(num_swdge_queues=1)
    pv = nc.declare_dram_parameter("pv", [TOTAL + 1, 1], f32, isOutput=False)
    idxs = nc.declare_dram_parameter("idx", [P, COLS], i32, isOutput=False)
    # 16 f32 per partition = one full 64B DRAM line per partition: each SDMA
    # engine writes whole aligned lines, so completion receipts run in
    # parallel (a [128,1] store = 32B/engine sub-line RMWs whose receipts
    # serialize, ~6us).
    out = nc.declare_dram_parameter("out", [P, 16], f32, isOutput=True)

    it = nc.alloc_sbuf_tensor("it", [P, COLS], i32).ap()
    g = nc.alloc_sbuf_tensor("g", [P, COLS], f32).ap()
    sg = nc.alloc_sbuf_tensor("sg", [P, COLS], f32).ap()
    wide = nc.alloc_sbuf_tensor("wide", [P, 16], f32).ap()
    warm = nc.alloc_sbuf_tensor("warm", [P, 1], f32).ap()

    sem_idx = nc.alloc_semaphore("s_idx")
    sem_g = nc.alloc_semaphore("s_g")
    sem_acc = nc.alloc_semaphore("s_acc")
    sem_out = nc.alloc_semaphore("s_out")

    nc.sync.dma_start(out=it[:], in_=idxs[:]).then_inc(sem_idx, 16)

    # Scalar stream head: dummy activation so the auto-inserted sigmoid
    # table load executes immediately, overlapping the idx-load latency.
    nc.scalar.activation(
        out=warm[:], in_=sg[:, 0:1], func=mybir.ActivationFunctionType.Sigmoid
    )

    nc.gpsimd.wait_ge(sem_idx, 16)
    nc.gpsimd.indirect_dma_start(
        out=g[:],
        out_offset=None,
        in_=pv[:],
        in_offset=bass.IndirectOffsetOnAxis(ap=it[:], axis=0),
    ).then_inc(sem_g, 16)

    # Row sums land directly in column 0 of the 16-column store tile; the
    # other 15 columns ship as garbage so each SDMA engine writes whole
    # aligned 64B DRAM lines (parallel completion receipts).
    nc.scalar.wait_ge(sem_g, 16)
    nc.scalar.activation(
        out=sg[:],
        in_=g[:],
        func=mybir.ActivationFunctionType.Sigmoid,
        accum_out=wide[:, 0:1],
    ).then_inc(sem_acc, 1)
    # The sequencer dispatches HWDGE desc-gen without waiting for the ACT
    # datapath, so an explicit same-engine wait orders the store after the
    # accumulator writeback.
    nc.scalar.wait_ge(sem_acc, 1)
    # Fire-and-forget store: the sem update exists only because HWDGE codegen
    # requires sync info — no engine waits on it. Engine halt does not gate
    # ring drain (the next execution's preamble dma_reset does) and host
    # readback is far later. No sem_clears needed either: the BIR-lowering
    # preamble clears the kernel sem range every execution.
    nc.scalar.dma_start(out=out[:], in_=wide[:]).then_inc(sem_out, 16)

    nc.finalize()
    return nc


def kernel(predicted_values, rel_idx, e1_idx, e2_idx, labels):
    global _NC, LAST_RESULTS
    pv = np.ascontiguousarray(np.asarray(predicted_values, dtype=np.float32))
    rel = np.asarray(rel_idx, dtype=np.int64)
    e1 = np.asarray(e1_idx, dtype=np.int64)
    e2 = np.asarray(e2_idx, dtype=np.int64)
    lab = np.asarray(labels, dtype=np.int64)

    owner = rel // RPC
    local_flat = (rel % RPC) * (E * E) + e1 * E + e2  # < TOTAL, fits int32
    pos_mask = lab == 1

    pv_flat = pv.reshape(R * E * E)
    host_extra = 0.0   # sum of w*sig for overflow triplets (host-computed)
    correction = 0.0   # sum over cores of 0.5*(pad_pos_c - pad_neg_c)
    in_maps = []
    for c in range(NCORES):
        m = owner == c
        fpos = local_flat[m & pos_mask]
        fneg = local_flat[m & ~pos_mask]
        # Host fallback for any sign bucket exceeding its region capacity:
        # compute w*sigmoid(score) for the overflow triplets exactly.
        for fi, sgn in ((fpos, 1.0), (fneg, -1.0)):
            if fi.size > RCAP:
                of = fi[RCAP:] + c * TOTAL
                s = pv_flat[of].astype(np.float64)
                host_extra += sgn * float(np.sum(1.0 / (1.0 + np.exp(-s))))
        fpos = fpos[:RCAP]
        fneg = fneg[:RCAP]
        correction += 0.5 * ((RCAP - fpos.size) - (RCAP - fneg.size))

        idx2d = np.full((P, COLS), TOTAL, np.int32)  # pads gather the 0.0 slot
        idx2d[:HALF].reshape(-1)[: fpos.size] = fpos.astype(np.int32)
        idx2d[HALF:].reshape(-1)[: fneg.size] = fneg.astype(np.int32)

        shard = np.empty((TOTAL + 1, 1), np.float32)
        shard[:TOTAL, 0] = pv_flat[c * TOTAL : (c + 1) * TOTAL]
        shard[TOTAL, 0] = 0.0
        in_maps.append({"pv": shard, "idx": idx2d})

    if _NC is None:
        _NC = _build_nc()

    res = run_bass_kernel_spmd(
        _NC, in_maps, core_ids=list(range(NCORES)), trace=TRACE
    )
    LAST_RESULTS = res

    # out[p] = sum_cols sigmoid(score); partitions <HALF hold positives,
    # >=HALF hold negatives; pads contribute 0.5 each.
    # sum w*sig(s) = sum_{p<HALF} out[p] - sum_{p>=HALF} out[p]
    #               - 0.5*pad_pos + 0.5*pad_neg
    asig = host_extra - correction
    for c in range(NCORES):
        o = np.asarray(res.results[c]["out"], dtype=np.float64).reshape(P, 16)[:, 0]
        asig += float(o[:HALF].sum() - o[HALF:].sum())

    neg = float(np.sum(lab == 0))
    loss = -(neg + asig) / ((1.0 + neg) * float(N))
    return np.array([loss], dtype=np.float32)
